# revision 20
# baseline (speedup 1.0000x reference)
"""CPDecoding (embedding_lookup) Trainium2 kernel, v2.

out[n] = sum_c fz[c,n]*fy[c,n]*fx[c,n], where f* is a 1-D linear
interpolation (grid_sample, align_corners=True) of a (96, 512) line table
at per-point coordinates.

Strategy (8 cores, data-parallel over the N=4096*192 points):
  - Host: compute (i0, w) per point/axis; sort points by z-index and pack
    8 points per z-table row (one 512B gather descriptor serves 8 points),
    padded to a fixed slot count. z/x tables are fp16 [f0|delta|pad] 512B
    rows; the y table is 64x supersampled (interpolation baked in, 256B
    f0-only rows) so the y interp disappears from the device.
  - Device (per ~4096-point chunk, all engines overlapped):
      DMA   3 dma_gathers (z rows 1/8 pts, y+x rows 1/pt)      ~12.4us
      Act   w broadcast tiles (per-point w replicated over c)   ~5.4us
      DVE   z/x interp, 2 products, tree + reduce over c       ~12.6us
      Pool  SWDGE descriptor generation                         ~6.0us
  - Host: unpermute per-point sums to the original order.
Cost-model timing: 333us/core vs 571us staged baseline. y-supersampling
quantization adds ~0.8% rel err (budget 2e-2).
"""

import numpy as np

N_CORES = 8
N_TOTAL = 4096 * 192
N_CORE = N_TOTAL // N_CORES      # 98304 points per core
P = 128                          # partitions
G = 8                            # points per z-row group
F = 800                          # free slots per partition (padded)
S = P * F                        # 102400 padded point slots per core
C = 96                           # components
R = 512                          # table resolution
ELEM = 256                       # fp16 elements per z table row (512 bytes)
SS = 64                          # y/x table supersampling factor
SELEM = 128                      # fp16 elements per y/x table row (256 bytes)
CHUNK_F = 16                     # f-blocks per chunk
CHUNK_PTS = P * CHUNK_F          # 4096 points per chunk
N_CHUNKS = F // CHUNK_F          # 25
ZBLK = CHUNK_F // G              # z-row blocks per chunk (4)

_BUILT = None


def _build_nc():
    """Build the per-core Bass program (SPMD, identical on all cores)."""
    import concourse.bacc as bacc
    import concourse.tile as tile
    from concourse import mybir
    from concourse.library_config import mlp as lib_mlp

    dt = mybir.dt
    Alu = mybir.AluOpType
    Axis = mybir.AxisListType

    nc = bacc.Bacc("TRN2", target_bir_lowering=False, debug=False,
                   num_devices=N_CORES, num_swdge_queues=1)

    # host-prepared inputs
    w_d = nc.dram_tensor("w", [P, 2 * F], dt.float16,
                         kind="ExternalInput").ap()
    idxz_d = nc.dram_tensor("idxz", [P, F], dt.int16, kind="ExternalInput").ap()
    idxy_d = nc.dram_tensor("idxy", [P, F * 8], dt.int16,
                            kind="ExternalInput").ap()
    idxx_d = nc.dram_tensor("idxx", [P, F * 8], dt.int16,
                            kind="ExternalInput").ap()
    tblz = nc.dram_tensor("tblz", [R, ELEM], dt.float16,
                          kind="ExternalInput").ap()
    tbly = nc.dram_tensor("tbly", [R * SS, SELEM], dt.float16,
                          kind="ExternalInput").ap()
    tblx = nc.dram_tensor("tblx", [R, ELEM], dt.float16,
                          kind="ExternalInput").ap()
    out_d = nc.dram_tensor("out", [P, F], dt.float32, kind="ExternalOutput").ap()

    with tile.TileContext(nc) as tc:
        with tc.tile_pool(name="persist", bufs=1) as pp:
            w_all = pp.tile([P, 2, F], dt.float16, tag="w")
            nc.sync.dma_start(w_all[:], w_d)
            idx_z = pp.tile([P, F], dt.int16, tag="iz")
            nc.sync.dma_start(idx_z[:], idxz_d)
            idx_y = pp.tile([P, F * 8], dt.int16, tag="iy")
            nc.sync.dma_start(idx_y[:], idxy_d)
            idx_x = pp.tile([P, F * 8], dt.int16, tag="ix")
            nc.sync.dma_start(idx_x[:], idxx_d)
            out_full = pp.tile([P, F], dt.float32, tag="out")

            with (
                tc.tile_pool(name="gath", bufs=2) as gp,
                tc.tile_pool(name="wt", bufs=2) as wp,
                tc.tile_pool(name="mid", bufs=2) as mp,
            ):
                with tc.tile_critical():
                    nc.gpsimd.load_library(lib_mlp)
                for c in range(N_CHUNKS):
                    fs = CHUNK_F * c

                    # --- gathers (rows: [f0(96) | delta(96) | pad]) ---
                    # idx tiles are wrapped-16 ([16, n/16] per band, replicated
                    # to all 8 bands); chunk c uses its 16-wrapped column slice
                    zc = ZBLK * P // 16              # 32 idx cols per chunk
                    gz = gp.tile([P, ZBLK, ELEM], dt.float16, tag="gz")
                    nc.gpsimd.dma_gather(
                        gz[:], tblz, idx_z[:, zc * c:zc * (c + 1)],
                        ZBLK * P, ZBLK * P, ELEM, elem_step=ELEM,
                        queue_num=0, single_packet=False)
                    yc = CHUNK_PTS // 16             # 256 idx cols per chunk
                    gy = gp.tile([P, CHUNK_F, SELEM], dt.float16, tag="gy")
                    nc.gpsimd.dma_gather(
                        gy[:], tbly, idx_y[:, yc * c:yc * (c + 1)],
                        CHUNK_PTS, CHUNK_PTS, SELEM, elem_step=SELEM,
                        queue_num=0, single_packet=False)
                    gx = gp.tile([P, CHUNK_F, ELEM], dt.float16, tag="gx")
                    nc.gpsimd.dma_gather(
                        gx[:], tblx, idx_x[:, yc * c:yc * (c + 1)],
                        CHUNK_PTS, CHUNK_PTS, ELEM, elem_step=ELEM,
                        queue_num=0, single_packet=False)

                    # --- z/x weight broadcast tiles (Act engine) ---
                    wtz = wp.tile([P, CHUNK_F, C], dt.float16, tag="wtz")
                    nc.scalar.copy(wtz[:], w_all[:, 0, fs:fs + CHUNK_F]
                                   .unsqueeze(2).broadcast_to([P, CHUNK_F, C]))
                    wtx = wp.tile([P, CHUNK_F, C], dt.float16, tag="wtx")
                    nc.scalar.copy(wtx[:], w_all[:, 1, fs:fs + CHUNK_F]
                                   .unsqueeze(2).broadcast_to([P, CHUNK_F, C]))

                    # --- z interp (rows shared by groups of 8 points) ---
                    d_z = (gz[:, :, C:2 * C].unsqueeze(2)
                           .broadcast_to([P, ZBLK, G, C]))
                    f0_z = (gz[:, :, 0:C].unsqueeze(2)
                            .broadcast_to([P, ZBLK, G, C]))
                    fz = mp.tile([P, CHUNK_F, C], dt.float16, tag="fz")
                    fz4 = fz[:].rearrange("p (q g) v -> p q g v", g=G)
                    wt04 = wtz[:].rearrange("p (q g) v -> p q g v", g=G)
                    nc.vector.tensor_mul(fz4, d_z, wt04)
                    nc.vector.tensor_add(fz4, fz4, f0_z)

                    # --- x interp (DVE) ---
                    fx = mp.tile([P, CHUNK_F, C], dt.float16, tag="fx")
                    nc.vector.tensor_mul(fx[:], gx[:, :, C:2 * C], wtx[:])
                    nc.vector.tensor_add(fx[:], fx[:], gx[:, :, 0:C])

                    # --- products (y rows are supersampled, direct) (DVE) ---
                    nc.vector.tensor_mul(fz[:], fz[:], gy[:, :, 0:C])
                    nc.vector.tensor_mul(fz[:], fz[:], fx[:])
                    # binary-tree halvings at tensor_tensor 2x rate, then a
                    # short tensor_reduce tail (reduce gets no DVE perf mode)
                    half = C
                    while half >= 12:
                        half //= 2
                        nc.vector.tensor_add(fz[:, :, 0:half],
                                             fz[:, :, 0:half],
                                             fz[:, :, half:2 * half])
                    nc.vector.reduce_sum(out_full[:, fs:fs + CHUNK_F],
                                         fz[:, :, 0:half], axis=Axis.X)

                nc.sync.dma_start(out_d, out_full[:])

    nc.compile()
    return nc


def _host_prep(in_tensor, line_z, line_y, line_x):
    """Build per-core input maps; returns (in_maps, per-core unsort perms)."""
    pts = np.ascontiguousarray(in_tensor.reshape(-1, 3).astype(np.float32))

    # z table: [f0(96) | delta(96) | pad] rows of 512B
    Lz = np.asarray(line_z, dtype=np.float32)
    z0 = Lz.T                                        # (512, 96)
    z1 = np.concatenate([Lz.T[1:], Lz.T[-1:]], axis=0)
    tbl_z = np.zeros((R, ELEM), dtype=np.float16)
    tbl_z[:, 0:C] = z0.astype(np.float16)
    tbl_z[:, C:2 * C] = (z1 - z0).astype(np.float16)

    # y/x tables: 64x supersampled, interpolation baked in, f0-only 256B rows
    def supersample(L):
        Lf = np.asarray(L, dtype=np.float32).T       # (512, 96)
        f0 = Lf
        f1 = np.concatenate([Lf[1:], Lf[-1:]], axis=0)
        r = (np.arange(SS, dtype=np.float32) / SS)[None, :, None]
        fine = f0[:, None, :] * (1.0 - r) + f1[:, None, :] * r
        row = np.zeros((R * SS, SELEM), dtype=np.float16)
        row[:, 0:C] = fine.reshape(R * SS, C).astype(np.float16)
        return row
    tbl_y = supersample(line_y)
    Lx = np.asarray(line_x, dtype=np.float32)
    x0 = Lx.T
    x1 = np.concatenate([Lx.T[1:], Lx.T[-1:]], axis=0)
    tbl_x = np.zeros((R, ELEM), dtype=np.float16)
    tbl_x[:, 0:C] = x0.astype(np.float16)
    tbl_x[:, C:2 * C] = (x1 - x0).astype(np.float16)

    # per-point indices/weights, axes ordered [z, y, x] = cols [2, 1, 0]
    pos = (pts + 1.0) * 0.5 * (R - 1)
    i0 = np.clip(np.floor(pos), 0, R - 1).astype(np.int32)
    w = (pos - i0).astype(np.float16)
    # supersampled y/x indices (nearest of the 64x grid)
    isup = np.clip(np.round(pos * SS), 0, (R - 1) * SS).astype(np.int32)

    def wrap16(flat):
        """j-ordered descriptor index list -> [16, n/16] band, replicated
        to all 8 16-partition bands."""
        w16 = flat.reshape(-1, 16).T
        return np.ascontiguousarray(np.tile(w16, (8, 1)))

    in_maps = []
    perms = []
    for k in range(N_CORES):
        sl = slice(k * N_CORE, (k + 1) * N_CORE)
        iz = i0[sl, 2]
        iy, ix = isup[sl, 1], i0[sl, 0]
        wz, wx = w[sl, 2], w[sl, 0]

        # sort by z-index; emit fixed-size groups of G per z-bin (padded)
        order = np.argsort(iz, kind="stable")
        izs = iz[order]
        # position of each sorted point within its z-bin
        binpos = np.arange(N_CORE) - np.searchsorted(izs, izs, side="left")
        ggid = binpos // G                            # group within bin
        key = izs.astype(np.int64) * 4096 + ggid      # global (bin, group)
        uniq, ginv = np.unique(key, return_inverse=True)
        n_groups = len(uniq)
        assert n_groups * G <= S, f"padding overflow: {n_groups * G} > {S}"
        slot_in_g = binpos % G
        # group g occupies partition g%128, free blocks (g//128)*G + m
        part = (ginv % P).astype(np.int32)
        free = ((ginv // P) * G + slot_in_g).astype(np.int32)

        # z-row per group, one descriptor per group, j == g ordering
        zrow = np.zeros(S // G, dtype=np.int16)
        zrow[:n_groups] = (uniq // 4096).astype(np.int16)

        # per-slot w / y / x arrays in (partition, free) layout
        w_arr = np.zeros((P, 2, F), dtype=np.float16)
        iy_arr = np.zeros((P, F), dtype=np.int16)
        ix_arr = np.zeros((P, F), dtype=np.int16)
        w_arr[part, 0, free] = wz[order]
        w_arr[part, 1, free] = wx[order]
        iy_arr[part, free] = iy[order].astype(np.int16)
        ix_arr[part, free] = ix[order].astype(np.int16)

        in_maps.append({
            "w": w_arr.reshape(P, 2 * F),
            "idxz": wrap16(zrow).reshape(P, F),
            "idxy": wrap16(iy_arr.T.reshape(-1)).reshape(P, F * 8),
            "idxx": wrap16(ix_arr.T.reshape(-1)).reshape(P, F * 8),
            "tblz": tbl_z,
            "tbly": tbl_y,
            "tblx": tbl_x,
        })
        # inverse mapping: sorted order + slot coordinates
        perms.append((order, part, free))
    return in_maps, perms


def _unshard(results, perms):
    outs = []
    for k in range(N_CORES):
        wv = np.asarray(results[k]["out"])           # (P, F)
        order, part, free = perms[k]
        vals = wv[part, free]                        # sorted-point order
        o = np.empty(N_CORE, dtype=np.float32)
        o[order] = vals
        outs.append(o)
    return np.concatenate(outs).reshape(4096, 192).astype(np.float32)


def kernel(in_tensor, line_z, line_y, line_x):
    global _BUILT
    from concourse.bass_utils import run_bass_kernel_spmd

    if _BUILT is None:
        _BUILT = _build_nc()
    nc = _BUILT
    in_maps, perms = _host_prep(np.asarray(in_tensor), np.asarray(line_z),
                                np.asarray(line_y), np.asarray(line_x))
    res = run_bass_kernel_spmd(nc, in_maps, list(range(N_CORES)))
    return _unshard(res.results, perms)


# revision 21
# speedup vs baseline: 1.0376x; 1.0376x over previous
"""CPDecoding (embedding_lookup) Trainium2 kernel, v2.

out[n] = sum_c fz[c,n]*fy[c,n]*fx[c,n], where f* is a 1-D linear
interpolation (grid_sample, align_corners=True) of a (96, 512) line table
at per-point coordinates.

Strategy (8 cores, data-parallel over the N=4096*192 points):
  - Host: compute (i0, w) per point/axis; sort points by z-index and pack
    8 points per z-table row (one 512B gather descriptor serves 8 points),
    padded to a fixed slot count. z/x tables are fp16 [f0|delta|pad] 512B
    rows; the y table is 64x supersampled (interpolation baked in, 256B
    f0-only rows) so the y interp disappears from the device.
  - Device (per ~4096-point chunk, all engines overlapped):
      DMA   3 dma_gathers (z rows 1/8 pts, y+x rows 1/pt)      ~12.4us
      Act   w broadcast tiles (per-point w replicated over c)   ~5.4us
      DVE   z/x interp, 2 products, tree + reduce over c       ~12.6us
      Pool  SWDGE descriptor generation                         ~6.0us
  - Host: unpermute per-point sums to the original order.
Cost-model timing: 333us/core vs 571us staged baseline. y-supersampling
quantization adds ~0.8% rel err (budget 2e-2).
"""

import numpy as np

N_CORES = 8
N_TOTAL = 4096 * 192
N_CORE = N_TOTAL // N_CORES      # 98304 points per core
P = 128                          # partitions
G = 8                            # points per z-row group
F = 800                          # free slots per partition (padded)
S = P * F                        # 102400 padded point slots per core
C = 96                           # components
R = 512                          # table resolution
ELEM = 256                       # fp16 elements per z table row (512 bytes)
SS = 64                          # y/x table supersampling factor
SELEM = 128                      # fp16 elements per y/x table row (256 bytes)
CHUNK_F = 40                     # f-blocks per chunk
CHUNK_PTS = P * CHUNK_F          # 4096 points per chunk
N_CHUNKS = F // CHUNK_F          # 25
ZBLK = CHUNK_F // G              # z-row blocks per chunk (4)

_BUILT = None


def _build_nc():
    """Build the per-core Bass program (SPMD, identical on all cores)."""
    import concourse.bacc as bacc
    import concourse.tile as tile
    from concourse import mybir
    from concourse.library_config import mlp as lib_mlp

    dt = mybir.dt
    Alu = mybir.AluOpType
    Axis = mybir.AxisListType

    nc = bacc.Bacc("TRN2", target_bir_lowering=False, debug=False,
                   num_devices=N_CORES, num_swdge_queues=1)

    # host-prepared inputs
    w_d = nc.dram_tensor("w", [P, 2 * F], dt.float16,
                         kind="ExternalInput").ap()
    idxz_d = nc.dram_tensor("idxz", [P, F], dt.int16, kind="ExternalInput").ap()
    idxy_d = nc.dram_tensor("idxy", [P, F * 8], dt.int16,
                            kind="ExternalInput").ap()
    idxx_d = nc.dram_tensor("idxx", [P, F * 8], dt.int16,
                            kind="ExternalInput").ap()
    tblz = nc.dram_tensor("tblz", [R, ELEM], dt.float16,
                          kind="ExternalInput").ap()
    tbly = nc.dram_tensor("tbly", [R * SS, SELEM], dt.float16,
                          kind="ExternalInput").ap()
    tblx = nc.dram_tensor("tblx", [R, ELEM], dt.float16,
                          kind="ExternalInput").ap()
    out_d = nc.dram_tensor("out", [P, F], dt.float32, kind="ExternalOutput").ap()

    with tile.TileContext(nc) as tc:
        with tc.tile_pool(name="persist", bufs=1) as pp:
            w_all = pp.tile([P, 2, F], dt.float16, tag="w")
            nc.sync.dma_start(w_all[:], w_d)
            idx_z = pp.tile([P, F], dt.int16, tag="iz")
            nc.sync.dma_start(idx_z[:], idxz_d)
            idx_y = pp.tile([P, F * 8], dt.int16, tag="iy")
            nc.sync.dma_start(idx_y[:], idxy_d)
            idx_x = pp.tile([P, F * 8], dt.int16, tag="ix")
            nc.sync.dma_start(idx_x[:], idxx_d)
            out_full = pp.tile([P, F], dt.float32, tag="out")

            with (
                tc.tile_pool(name="gath", bufs=2) as gp,
                tc.tile_pool(name="wt", bufs=2) as wp,
                tc.tile_pool(name="mid", bufs=2) as mp,
            ):
                with tc.tile_critical():
                    nc.gpsimd.load_library(lib_mlp)
                for c in range(N_CHUNKS):
                    fs = CHUNK_F * c

                    # --- gathers (rows: [f0(96) | delta(96) | pad]) ---
                    # idx tiles are wrapped-16 ([16, n/16] per band, replicated
                    # to all 8 bands); chunk c uses its 16-wrapped column slice
                    zc = ZBLK * P // 16              # 32 idx cols per chunk
                    gz = gp.tile([P, ZBLK, ELEM], dt.float16, tag="gz")
                    nc.gpsimd.dma_gather(
                        gz[:], tblz, idx_z[:, zc * c:zc * (c + 1)],
                        ZBLK * P, ZBLK * P, ELEM, elem_step=ELEM,
                        queue_num=0, single_packet=False)
                    yc = CHUNK_PTS // 16             # 256 idx cols per chunk
                    gy = gp.tile([P, CHUNK_F, SELEM], dt.float16, tag="gy")
                    nc.gpsimd.dma_gather(
                        gy[:], tbly, idx_y[:, yc * c:yc * (c + 1)],
                        CHUNK_PTS, CHUNK_PTS, SELEM, elem_step=SELEM,
                        queue_num=0, single_packet=False)
                    gx = gp.tile([P, CHUNK_F, ELEM], dt.float16, tag="gx")
                    nc.gpsimd.dma_gather(
                        gx[:], tblx, idx_x[:, yc * c:yc * (c + 1)],
                        CHUNK_PTS, CHUNK_PTS, ELEM, elem_step=ELEM,
                        queue_num=0, single_packet=False)

                    # --- z/x weight broadcast tiles (Act engine) ---
                    wtz = wp.tile([P, CHUNK_F, C], dt.float16, tag="wtz")
                    nc.scalar.copy(wtz[:], w_all[:, 0, fs:fs + CHUNK_F]
                                   .unsqueeze(2).broadcast_to([P, CHUNK_F, C]))
                    wtx = wp.tile([P, CHUNK_F, C], dt.float16, tag="wtx")
                    nc.scalar.copy(wtx[:], w_all[:, 1, fs:fs + CHUNK_F]
                                   .unsqueeze(2).broadcast_to([P, CHUNK_F, C]))

                    # --- z interp (rows shared by groups of 8 points) ---
                    d_z = (gz[:, :, C:2 * C].unsqueeze(2)
                           .broadcast_to([P, ZBLK, G, C]))
                    f0_z = (gz[:, :, 0:C].unsqueeze(2)
                            .broadcast_to([P, ZBLK, G, C]))
                    fz = mp.tile([P, CHUNK_F, C], dt.float16, tag="fz")
                    fz4 = fz[:].rearrange("p (q g) v -> p q g v", g=G)
                    wt04 = wtz[:].rearrange("p (q g) v -> p q g v", g=G)
                    nc.vector.tensor_mul(fz4, d_z, wt04)
                    nc.vector.tensor_add(fz4, fz4, f0_z)

                    # --- x interp (DVE) ---
                    fx = mp.tile([P, CHUNK_F, C], dt.float16, tag="fx")
                    nc.vector.tensor_mul(fx[:], gx[:, :, C:2 * C], wtx[:])
                    nc.vector.tensor_add(fx[:], fx[:], gx[:, :, 0:C])

                    # --- products (y rows are supersampled, direct) (DVE) ---
                    nc.vector.tensor_mul(fz[:], fz[:], gy[:, :, 0:C])
                    nc.vector.tensor_mul(fz[:], fz[:], fx[:])
                    # binary-tree halvings at tensor_tensor 2x rate, then a
                    # short tensor_reduce tail (reduce gets no DVE perf mode)
                    half = C
                    while half >= 12:
                        half //= 2
                        nc.vector.tensor_add(fz[:, :, 0:half],
                                             fz[:, :, 0:half],
                                             fz[:, :, half:2 * half])
                    nc.vector.reduce_sum(out_full[:, fs:fs + CHUNK_F],
                                         fz[:, :, 0:half], axis=Axis.X)

                nc.sync.dma_start(out_d, out_full[:])

    nc.compile()
    return nc


def _host_prep(in_tensor, line_z, line_y, line_x):
    """Build per-core input maps; returns (in_maps, per-core unsort perms)."""
    pts = np.ascontiguousarray(in_tensor.reshape(-1, 3).astype(np.float32))

    # z table: [f0(96) | delta(96) | pad] rows of 512B
    Lz = np.asarray(line_z, dtype=np.float32)
    z0 = Lz.T                                        # (512, 96)
    z1 = np.concatenate([Lz.T[1:], Lz.T[-1:]], axis=0)
    tbl_z = np.zeros((R, ELEM), dtype=np.float16)
    tbl_z[:, 0:C] = z0.astype(np.float16)
    tbl_z[:, C:2 * C] = (z1 - z0).astype(np.float16)

    # y/x tables: 64x supersampled, interpolation baked in, f0-only 256B rows
    def supersample(L):
        Lf = np.asarray(L, dtype=np.float32).T       # (512, 96)
        f0 = Lf
        f1 = np.concatenate([Lf[1:], Lf[-1:]], axis=0)
        r = (np.arange(SS, dtype=np.float32) / SS)[None, :, None]
        fine = f0[:, None, :] * (1.0 - r) + f1[:, None, :] * r
        row = np.zeros((R * SS, SELEM), dtype=np.float16)
        row[:, 0:C] = fine.reshape(R * SS, C).astype(np.float16)
        return row
    tbl_y = supersample(line_y)
    Lx = np.asarray(line_x, dtype=np.float32)
    x0 = Lx.T
    x1 = np.concatenate([Lx.T[1:], Lx.T[-1:]], axis=0)
    tbl_x = np.zeros((R, ELEM), dtype=np.float16)
    tbl_x[:, 0:C] = x0.astype(np.float16)
    tbl_x[:, C:2 * C] = (x1 - x0).astype(np.float16)

    # per-point indices/weights, axes ordered [z, y, x] = cols [2, 1, 0]
    pos = (pts + 1.0) * 0.5 * (R - 1)
    i0 = np.clip(np.floor(pos), 0, R - 1).astype(np.int32)
    w = (pos - i0).astype(np.float16)
    # supersampled y/x indices (nearest of the 64x grid)
    isup = np.clip(np.round(pos * SS), 0, (R - 1) * SS).astype(np.int32)

    def wrap16(flat):
        """j-ordered descriptor index list -> [16, n/16] band, replicated
        to all 8 16-partition bands."""
        w16 = flat.reshape(-1, 16).T
        return np.ascontiguousarray(np.tile(w16, (8, 1)))

    in_maps = []
    perms = []
    for k in range(N_CORES):
        sl = slice(k * N_CORE, (k + 1) * N_CORE)
        iz = i0[sl, 2]
        iy, ix = isup[sl, 1], i0[sl, 0]
        wz, wx = w[sl, 2], w[sl, 0]

        # sort by z-index; emit fixed-size groups of G per z-bin (padded)
        order = np.argsort(iz, kind="stable")
        izs = iz[order]
        # position of each sorted point within its z-bin
        binpos = np.arange(N_CORE) - np.searchsorted(izs, izs, side="left")
        ggid = binpos // G                            # group within bin
        key = izs.astype(np.int64) * 4096 + ggid      # global (bin, group)
        uniq, ginv = np.unique(key, return_inverse=True)
        n_groups = len(uniq)
        assert n_groups * G <= S, f"padding overflow: {n_groups * G} > {S}"
        slot_in_g = binpos % G
        # group g occupies partition g%128, free blocks (g//128)*G + m
        part = (ginv % P).astype(np.int32)
        free = ((ginv // P) * G + slot_in_g).astype(np.int32)

        # z-row per group, one descriptor per group, j == g ordering
        zrow = np.zeros(S // G, dtype=np.int16)
        zrow[:n_groups] = (uniq // 4096).astype(np.int16)

        # per-slot w / y / x arrays in (partition, free) layout
        w_arr = np.zeros((P, 2, F), dtype=np.float16)
        iy_arr = np.zeros((P, F), dtype=np.int16)
        ix_arr = np.zeros((P, F), dtype=np.int16)
        w_arr[part, 0, free] = wz[order]
        w_arr[part, 1, free] = wx[order]
        iy_arr[part, free] = iy[order].astype(np.int16)
        ix_arr[part, free] = ix[order].astype(np.int16)

        in_maps.append({
            "w": w_arr.reshape(P, 2 * F),
            "idxz": wrap16(zrow).reshape(P, F),
            "idxy": wrap16(iy_arr.T.reshape(-1)).reshape(P, F * 8),
            "idxx": wrap16(ix_arr.T.reshape(-1)).reshape(P, F * 8),
            "tblz": tbl_z,
            "tbly": tbl_y,
            "tblx": tbl_x,
        })
        # inverse mapping: sorted order + slot coordinates
        perms.append((order, part, free))
    return in_maps, perms


def _unshard(results, perms):
    outs = []
    for k in range(N_CORES):
        wv = np.asarray(results[k]["out"])           # (P, F)
        order, part, free = perms[k]
        vals = wv[part, free]                        # sorted-point order
        o = np.empty(N_CORE, dtype=np.float32)
        o[order] = vals
        outs.append(o)
    return np.concatenate(outs).reshape(4096, 192).astype(np.float32)


def kernel(in_tensor, line_z, line_y, line_x):
    global _BUILT
    from concourse.bass_utils import run_bass_kernel_spmd

    if _BUILT is None:
        _BUILT = _build_nc()
    nc = _BUILT
    in_maps, perms = _host_prep(np.asarray(in_tensor), np.asarray(line_z),
                                np.asarray(line_y), np.asarray(line_x))
    res = run_bass_kernel_spmd(nc, in_maps, list(range(N_CORES)))
    return _unshard(res.results, perms)


# revision 22
# speedup vs baseline: 1.0456x; 1.0077x over previous
"""CPDecoding (embedding_lookup) Trainium2 kernel, v2.

out[n] = sum_c fz[c,n]*fy[c,n]*fx[c,n], where f* is a 1-D linear
interpolation (grid_sample, align_corners=True) of a (96, 512) line table
at per-point coordinates.

Strategy (8 cores, data-parallel over the N=4096*192 points):
  - Host: compute (i0, w) per point/axis; sort points by z-index and pack
    8 points per z-table row (one 512B gather descriptor serves 8 points),
    padded to a fixed slot count. z/x tables are fp16 [f0|delta|pad] 512B
    rows; the y table is 64x supersampled (interpolation baked in, 256B
    f0-only rows) so the y interp disappears from the device.
  - Device (per ~4096-point chunk, all engines overlapped):
      DMA   3 dma_gathers (z rows 1/8 pts, y+x rows 1/pt)      ~12.4us
      Act   w broadcast tiles (per-point w replicated over c)   ~5.4us
      DVE   z/x interp, 2 products, tree + reduce over c       ~12.6us
      Pool  SWDGE descriptor generation                         ~6.0us
  - Host: unpermute per-point sums to the original order.
Cost-model timing: 333us/core vs 571us staged baseline. y-supersampling
quantization adds ~0.8% rel err (budget 2e-2).
"""

import numpy as np

N_CORES = 8
N_TOTAL = 4096 * 192
N_CORE = N_TOTAL // N_CORES      # 98304 points per core
P = 128                          # partitions
G = 8                            # points per z-row group
F = 800                          # free slots per partition (padded)
S = P * F                        # 102400 padded point slots per core
C = 96                           # components
R = 512                          # table resolution
ELEM = 256                       # fp16 elements per z table row (512 bytes)
SS = 64                          # y/x table supersampling factor
SELEM = 128                      # fp16 elements per y/x table row (256 bytes)
CHUNK_F = 32                     # f-blocks per chunk
CHUNK_PTS = P * CHUNK_F          # 4096 points per chunk
N_CHUNKS = F // CHUNK_F          # 25
ZBLK = CHUNK_F // G              # z-row blocks per chunk (4)

_BUILT = None


def _build_nc():
    """Build the per-core Bass program (SPMD, identical on all cores)."""
    import concourse.bacc as bacc
    import concourse.tile as tile
    from concourse import mybir
    from concourse.library_config import mlp as lib_mlp

    dt = mybir.dt
    Alu = mybir.AluOpType
    Axis = mybir.AxisListType

    nc = bacc.Bacc("TRN2", target_bir_lowering=False, debug=False,
                   num_devices=N_CORES, num_swdge_queues=1)

    # host-prepared inputs
    w_d = nc.dram_tensor("w", [P, 2 * F], dt.float16,
                         kind="ExternalInput").ap()
    idxz_d = nc.dram_tensor("idxz", [P, F], dt.int16, kind="ExternalInput").ap()
    idxy_d = nc.dram_tensor("idxy", [P, F * 8], dt.int16,
                            kind="ExternalInput").ap()
    idxx_d = nc.dram_tensor("idxx", [P, F * 8], dt.int16,
                            kind="ExternalInput").ap()
    tblz = nc.dram_tensor("tblz", [R, ELEM], dt.float16,
                          kind="ExternalInput").ap()
    tbly = nc.dram_tensor("tbly", [R * SS, SELEM], dt.float16,
                          kind="ExternalInput").ap()
    tblx = nc.dram_tensor("tblx", [R, ELEM], dt.float16,
                          kind="ExternalInput").ap()
    out_d = nc.dram_tensor("out", [P, F], dt.float32, kind="ExternalOutput").ap()

    with tile.TileContext(nc) as tc:
        with tc.tile_pool(name="persist", bufs=1) as pp:
            w_all = pp.tile([P, 2, F], dt.float16, tag="w")
            nc.sync.dma_start(w_all[:], w_d)
            idx_z = pp.tile([P, F], dt.int16, tag="iz")
            nc.sync.dma_start(idx_z[:], idxz_d)
            idx_y = pp.tile([P, F * 8], dt.int16, tag="iy")
            nc.sync.dma_start(idx_y[:], idxy_d)
            idx_x = pp.tile([P, F * 8], dt.int16, tag="ix")
            nc.sync.dma_start(idx_x[:], idxx_d)
            out_full = pp.tile([P, F], dt.float32, tag="out")

            with (
                tc.tile_pool(name="gath", bufs=2) as gp,
                tc.tile_pool(name="wt", bufs=2) as wp,
                tc.tile_pool(name="mid", bufs=2) as mp,
            ):
                with tc.tile_critical():
                    nc.gpsimd.load_library(lib_mlp)
                for c in range(N_CHUNKS):
                    fs = CHUNK_F * c

                    # --- gathers (rows: [f0(96) | delta(96) | pad]) ---
                    # idx tiles are wrapped-16 ([16, n/16] per band, replicated
                    # to all 8 bands); chunk c uses its 16-wrapped column slice
                    zc = ZBLK * P // 16              # 32 idx cols per chunk
                    gz = gp.tile([P, ZBLK, ELEM], dt.float16, tag="gz")
                    nc.gpsimd.dma_gather(
                        gz[:], tblz, idx_z[:, zc * c:zc * (c + 1)],
                        ZBLK * P, ZBLK * P, ELEM, elem_step=ELEM,
                        queue_num=0, single_packet=False)
                    yc = CHUNK_PTS // 16             # 256 idx cols per chunk
                    gy = gp.tile([P, CHUNK_F, SELEM], dt.float16, tag="gy")
                    nc.gpsimd.dma_gather(
                        gy[:], tbly, idx_y[:, yc * c:yc * (c + 1)],
                        CHUNK_PTS, CHUNK_PTS, SELEM, elem_step=SELEM,
                        queue_num=0, single_packet=False)
                    gx = gp.tile([P, CHUNK_F, ELEM], dt.float16, tag="gx")
                    nc.gpsimd.dma_gather(
                        gx[:], tblx, idx_x[:, yc * c:yc * (c + 1)],
                        CHUNK_PTS, CHUNK_PTS, ELEM, elem_step=ELEM,
                        queue_num=0, single_packet=False)

                    # --- z/x weight broadcast tiles (Act engine) ---
                    wtz = wp.tile([P, CHUNK_F, C], dt.float16, tag="wtz")
                    nc.scalar.copy(wtz[:], w_all[:, 0, fs:fs + CHUNK_F]
                                   .unsqueeze(2).broadcast_to([P, CHUNK_F, C]))
                    wtx = wp.tile([P, CHUNK_F, C], dt.float16, tag="wtx")
                    nc.scalar.copy(wtx[:], w_all[:, 1, fs:fs + CHUNK_F]
                                   .unsqueeze(2).broadcast_to([P, CHUNK_F, C]))

                    # --- z interp (rows shared by groups of 8 points) ---
                    d_z = (gz[:, :, C:2 * C].unsqueeze(2)
                           .broadcast_to([P, ZBLK, G, C]))
                    f0_z = (gz[:, :, 0:C].unsqueeze(2)
                            .broadcast_to([P, ZBLK, G, C]))
                    fz = mp.tile([P, CHUNK_F, C], dt.float16, tag="fz")
                    fz4 = fz[:].rearrange("p (q g) v -> p q g v", g=G)
                    wt04 = wtz[:].rearrange("p (q g) v -> p q g v", g=G)
                    nc.vector.tensor_mul(fz4, d_z, wt04)
                    nc.vector.tensor_add(fz4, fz4, f0_z)

                    # --- x interp (DVE) ---
                    fx = mp.tile([P, CHUNK_F, C], dt.float16, tag="fx")
                    nc.vector.tensor_mul(fx[:], gx[:, :, C:2 * C], wtx[:])
                    nc.vector.tensor_add(fx[:], fx[:], gx[:, :, 0:C])

                    # --- products (y rows are supersampled, direct) (DVE) ---
                    nc.vector.tensor_mul(fz[:], fz[:], gy[:, :, 0:C])
                    nc.vector.tensor_mul(fz[:], fz[:], fx[:])
                    # binary-tree halvings at tensor_tensor 2x rate, then a
                    # short tensor_reduce tail (reduce gets no DVE perf mode)
                    half = C
                    while half >= 12:
                        half //= 2
                        nc.vector.tensor_add(fz[:, :, 0:half],
                                             fz[:, :, 0:half],
                                             fz[:, :, half:2 * half])
                    nc.vector.reduce_sum(out_full[:, fs:fs + CHUNK_F],
                                         fz[:, :, 0:half], axis=Axis.X)

                nc.sync.dma_start(out_d, out_full[:])

    nc.compile()
    return nc


def _host_prep(in_tensor, line_z, line_y, line_x):
    """Build per-core input maps; returns (in_maps, per-core unsort perms)."""
    pts = np.ascontiguousarray(in_tensor.reshape(-1, 3).astype(np.float32))

    # z table: [f0(96) | delta(96) | pad] rows of 512B
    Lz = np.asarray(line_z, dtype=np.float32)
    z0 = Lz.T                                        # (512, 96)
    z1 = np.concatenate([Lz.T[1:], Lz.T[-1:]], axis=0)
    tbl_z = np.zeros((R, ELEM), dtype=np.float16)
    tbl_z[:, 0:C] = z0.astype(np.float16)
    tbl_z[:, C:2 * C] = (z1 - z0).astype(np.float16)

    # y/x tables: 64x supersampled, interpolation baked in, f0-only 256B rows
    def supersample(L):
        Lf = np.asarray(L, dtype=np.float32).T       # (512, 96)
        f0 = Lf
        f1 = np.concatenate([Lf[1:], Lf[-1:]], axis=0)
        r = (np.arange(SS, dtype=np.float32) / SS)[None, :, None]
        fine = f0[:, None, :] * (1.0 - r) + f1[:, None, :] * r
        row = np.zeros((R * SS, SELEM), dtype=np.float16)
        row[:, 0:C] = fine.reshape(R * SS, C).astype(np.float16)
        return row
    tbl_y = supersample(line_y)
    Lx = np.asarray(line_x, dtype=np.float32)
    x0 = Lx.T
    x1 = np.concatenate([Lx.T[1:], Lx.T[-1:]], axis=0)
    tbl_x = np.zeros((R, ELEM), dtype=np.float16)
    tbl_x[:, 0:C] = x0.astype(np.float16)
    tbl_x[:, C:2 * C] = (x1 - x0).astype(np.float16)

    # per-point indices/weights, axes ordered [z, y, x] = cols [2, 1, 0]
    pos = (pts + 1.0) * 0.5 * (R - 1)
    i0 = np.clip(np.floor(pos), 0, R - 1).astype(np.int32)
    w = (pos - i0).astype(np.float16)
    # supersampled y/x indices (nearest of the 64x grid)
    isup = np.clip(np.round(pos * SS), 0, (R - 1) * SS).astype(np.int32)

    def wrap16(flat):
        """j-ordered descriptor index list -> [16, n/16] band, replicated
        to all 8 16-partition bands."""
        w16 = flat.reshape(-1, 16).T
        return np.ascontiguousarray(np.tile(w16, (8, 1)))

    in_maps = []
    perms = []
    for k in range(N_CORES):
        sl = slice(k * N_CORE, (k + 1) * N_CORE)
        iz = i0[sl, 2]
        iy, ix = isup[sl, 1], i0[sl, 0]
        wz, wx = w[sl, 2], w[sl, 0]

        # sort by z-index; emit fixed-size groups of G per z-bin (padded)
        order = np.argsort(iz, kind="stable")
        izs = iz[order]
        # position of each sorted point within its z-bin
        binpos = np.arange(N_CORE) - np.searchsorted(izs, izs, side="left")
        ggid = binpos // G                            # group within bin
        key = izs.astype(np.int64) * 4096 + ggid      # global (bin, group)
        uniq, ginv = np.unique(key, return_inverse=True)
        n_groups = len(uniq)
        assert n_groups * G <= S, f"padding overflow: {n_groups * G} > {S}"
        slot_in_g = binpos % G
        # group g occupies partition g%128, free blocks (g//128)*G + m
        part = (ginv % P).astype(np.int32)
        free = ((ginv // P) * G + slot_in_g).astype(np.int32)

        # z-row per group, one descriptor per group, j == g ordering
        zrow = np.zeros(S // G, dtype=np.int16)
        zrow[:n_groups] = (uniq // 4096).astype(np.int16)

        # per-slot w / y / x arrays in (partition, free) layout
        w_arr = np.zeros((P, 2, F), dtype=np.float16)
        iy_arr = np.zeros((P, F), dtype=np.int16)
        ix_arr = np.zeros((P, F), dtype=np.int16)
        w_arr[part, 0, free] = wz[order]
        w_arr[part, 1, free] = wx[order]
        iy_arr[part, free] = iy[order].astype(np.int16)
        ix_arr[part, free] = ix[order].astype(np.int16)

        in_maps.append({
            "w": w_arr.reshape(P, 2 * F),
            "idxz": wrap16(zrow).reshape(P, F),
            "idxy": wrap16(iy_arr.T.reshape(-1)).reshape(P, F * 8),
            "idxx": wrap16(ix_arr.T.reshape(-1)).reshape(P, F * 8),
            "tblz": tbl_z,
            "tbly": tbl_y,
            "tblx": tbl_x,
        })
        # inverse mapping: sorted order + slot coordinates
        perms.append((order, part, free))
    return in_maps, perms


def _unshard(results, perms):
    outs = []
    for k in range(N_CORES):
        wv = np.asarray(results[k]["out"])           # (P, F)
        order, part, free = perms[k]
        vals = wv[part, free]                        # sorted-point order
        o = np.empty(N_CORE, dtype=np.float32)
        o[order] = vals
        outs.append(o)
    return np.concatenate(outs).reshape(4096, 192).astype(np.float32)


def kernel(in_tensor, line_z, line_y, line_x):
    global _BUILT
    from concourse.bass_utils import run_bass_kernel_spmd

    if _BUILT is None:
        _BUILT = _build_nc()
    nc = _BUILT
    in_maps, perms = _host_prep(np.asarray(in_tensor), np.asarray(line_z),
                                np.asarray(line_y), np.asarray(line_x))
    res = run_bass_kernel_spmd(nc, in_maps, list(range(N_CORES)))
    return _unshard(res.results, perms)
